# revision 1
# baseline (speedup 1.0000x reference)
"""Trainium2 Bass kernel for nn_Entropy (histogram_binning): per-pixel Shannon
entropy of a 5x5-window KDE histogram over 256 intensity bins.

Math (validated in f32 vs reference):
  k(x,b) = sigmoid'(10(x-b)) = 0.25*(1 - tanh^2(5x-5b))   [exact identity]
  q[h,w,b] = 5x5 window sum of k;  S = sum_b q;  p = q/(S+EPS)
  out = -sum_b p*ln(p+EPS) = -r * sum_b q*ln(r*q+EPS),  r = 1/(S+EPS)
  S comes analytically per pixel from 5 taps of the KDE kernel around
  frac(x) (range-masked), then a tiny 5x5 window sum.

Layout per (image, bin-half) stripe: partitions = h (96), free = (w, b).
  - d' = 5x - 5b on TensorE: K=9 matmuls; stationary = [5*x^T(8 w-rows);
    ones], moving = tiny shipped selector constant.
  - tanh on ScalarE (evacuates PSUM); k = 0.25 - 0.25 t^2 on VectorE.
  - H-window: banded-matrix matmul (TensorE) -> PSUM, evacuated by
    ScalarE into a w-inner padded stripe [b-block: 3 zero pads + 96 w].
  - W-window: one in-place VectorE prefix scan per stripe over the padded
    row; q[w,b] = P[99b+w+5] - P[99b+w] (pads absorb all edges).
  - backend per w: L = ln(r*q + EPS) on ScalarE (per-partition scale AP),
    e = q*L and QL = sum_b(e) on VectorE; E = -r*QL.

Stripes are software-pipelined (3 stripe buffers) so image i+1's front end
overlaps image i's backend. Sharding: B*C = 24 images, 3 per core across 8
cores; no collectives. Self-contained; compiled once per process.
"""

import sys

sys.path.insert(0, "/opt/trn_rl_repo")

import numpy as np

H = 96
W = 96
NB = 256
NBH = 128         # bins per stripe (half)
NIMG = 3
NCORES = 8
EPS = 1e-10
ZB = 99           # per-bin block in a stripe: 3 zero pads + 96 w cols
WQ = 8            # w rows per stationary group
NG = W // WQ      # 12 groups

_CACHE = {}


def _build_consts():
    # selector constants per bin-half: [9, WQ*NBH]; rows j=0..7 mark w-offset
    # j over that bin-block; row 8 = -5*b
    crhs = []
    for half in range(2):
        c = np.zeros((9, WQ * NBH), dtype=np.float32)
        for j in range(WQ):
            c[j, j * NBH:(j + 1) * NBH] = 1.0
        b = np.arange(NBH, dtype=np.float32) + half * NBH
        c[8, :] = np.tile(-5.0 * b, WQ)
        crhs.append(c)
    hh = np.arange(H)
    band = (np.abs(hh[:, None] - hh[None, :]) <= 2).astype(np.float32)
    return crhs[0], crhs[1], band


def _emit_kernel(nc, tc, ctx, ins, outs):
    from concourse import mybir

    f32 = mybir.dt.float32
    i32 = mybir.dt.int32
    AF = mybir.ActivationFunctionType
    OP = mybir.AluOpType

    x_d, xt_d, crhs0_d, crhs1_d, band_d = ins
    (ent_d,) = outs
    NW = NIMG * W

    consts = ctx.enter_context(tc.tile_pool(name="consts", bufs=1))
    stripes = ctx.enter_context(tc.tile_pool(name="stripes", bufs=2))
    sm = ctx.enter_context(tc.tile_pool(name="sm", bufs=1))
    chunks = ctx.enter_context(tc.tile_pool(name="chunks", bufs=2))
    psum = ctx.enter_context(tc.tile_pool(name="psum", bufs=4, space="PSUM"))

    # ---- constants / inputs ----
    crhs_sb = []
    for half, cd in ((0, crhs0_d), (1, crhs1_d)):
        t = consts.tile([73, WQ * NBH], f32, tag=f"crhs{half}")
        for k3 in range(3):
            nc.sync.dma_start(t[32 * k3:32 * k3 + 9, :], cd[:])
        crhs_sb.append(t)
    band_sb = consts.tile([H, H], f32)
    nc.sync.dma_start(band_sb[:], band_d[:])

    xall = consts.tile([H, NW], f32)
    xtall = consts.tile([W, NIMG * H], f32)
    for i in range(NIMG):
        nc.sync.dma_start(xall[:, i * W:(i + 1) * W], x_d[i])
        nc.sync.dma_start(xtall[:, i * H:(i + 1) * H], xt_d[i])

    ones_sb = consts.tile([1, NIMG * H], f32)
    nc.vector.memset(ones_sb[:], 1.0)
    xt5_all = consts.tile([W, NIMG * H], f32)
    nc.vector.tensor_scalar(xt5_all[:], xtall[:], 5.0, None, op0=OP.mult)
    # stationary groups [9 rows: 5*xT(8 w) ; ones], 3 per tile at bases 0/32/64
    xt9g = []
    for tg in range(4):
        gt = consts.tile([73, NIMG * H], f32, tag=f"xt9g{tg}")
        for k3 in range(3):
            g = tg * 3 + k3
            base = 32 * k3
            nc.sync.dma_start(gt[base:base + 8, :], xt5_all[8 * g:8 * g + 8, :])
            nc.sync.dma_start(gt[base + 8:base + 9, :], ones_sb[:])
            xt9g.append(gt[base:base + 9])

    bias_tiles = {}

    def bias_ap(val):
        if val not in bias_tiles:
            t = consts.tile([H, 1], f32, tag=f"bias{val}")
            nc.vector.memset(t[:], val)
            bias_tiles[val] = t
        return bias_tiles[val][:]

    # =====================  S path (tiny, [96, 288])  =====================
    ni = sm.tile([H, NW], i32)
    nc.vector.tensor_copy(ni[:], xall[:])
    nf = sm.tile([H, NW], f32)
    nc.vector.tensor_copy(nf[:], ni[:])
    u = sm.tile([H, NW], f32)
    nc.vector.tensor_tensor(u[:], xall[:], nf[:], op=OP.subtract)
    taps = (-2, -1, 0, 1, 2)
    sq = {}
    for o in taps:
        v = sm.tile([H, NW], f32, tag=f"v{o}")
        nc.scalar.activation(v[:], u[:], AF.Tanh, bias=bias_ap(-5.0 * o), scale=5.0)
        s2 = sm.tile([H, NW], f32, tag=f"sq{o}")
        nc.scalar.activation(s2[:], v[:], AF.Square)
        sq[o] = s2
    masks = {}
    for o in taps:
        if o == 0:
            continue
        m = sm.tile([H, NW], f32, tag=f"m{o}")
        if o < 0:
            nc.vector.tensor_scalar(m[:], nf[:], float(-o), None, op0=OP.is_ge)
        else:
            nc.vector.tensor_scalar(m[:], nf[:], float(255 - o), None, op0=OP.is_le)
        masks[o] = m
    cnt = sm.tile([H, NW], f32)
    nc.vector.tensor_tensor(cnt[:], masks[-2][:], masks[-1][:], op=OP.add)
    nc.vector.tensor_tensor(cnt[:], cnt[:], masks[1][:], op=OP.add)
    nc.vector.tensor_tensor(cnt[:], cnt[:], masks[2][:], op=OP.add)
    nc.vector.tensor_scalar(cnt[:], cnt[:], 1.0, None, op0=OP.add)
    ssum = sm.tile([H, NW], f32)
    nc.vector.tensor_copy(ssum[:], sq[0][:])
    for o in (-2, -1, 1, 2):
        t_m = sm.tile([H, NW], f32, tag=f"tm{o}")
        nc.vector.tensor_tensor(t_m[:], masks[o][:], sq[o][:], op=OP.mult)
        nc.vector.tensor_tensor(ssum[:], ssum[:], t_m[:], op=OP.add)
    spix = sm.tile([H, NW], f32)
    nc.vector.tensor_tensor(spix[:], cnt[:], ssum[:], op=OP.subtract)
    nc.vector.tensor_scalar(spix[:], spix[:], 0.25, None, op0=OP.mult)
    ps_s = psum.tile([H, 1024], f32, tag="ps")
    nc.tensor.matmul(ps_s[:, 0:NW], band_sb[:], spix[:], start=True, stop=True)
    sh = sm.tile([H, NW], f32)
    nc.scalar.copy(sh[:], ps_s[:, 0:NW])
    shp = sm.tile([H, NIMG, W + 4], f32)
    nc.vector.memset(shp[:], 0.0)
    for i in range(NIMG):
        nc.vector.tensor_copy(shp[:, i, 2:2 + W], sh[:, i * W:(i + 1) * W])
    swin = sm.tile([H, NIMG, W], f32)
    nc.vector.tensor_tensor(swin[:], shp[:, :, 0:W], shp[:, :, 1:1 + W], op=OP.add)
    for j in (2, 3, 4):
        nc.vector.tensor_tensor(swin[:], swin[:], shp[:, :, j:j + W], op=OP.add)
    rtile = sm.tile([H, NW], f32)
    sw_flat = swin[:].rearrange("p a b -> p (a b)")
    nc.vector.tensor_scalar(rtile[:], sw_flat, EPS, None, op0=OP.add)
    nc.vector.reciprocal(rtile[:], rtile[:])

    # =====================  main path: per (image, bin-half) stripe  ========
    QL = sm.tile([H, NW], f32)
    stripe_store = {}

    def emit_front(i, half):
        qh = stripes.tile([H, NBH * ZB + 8], f32, tag="qh")
        qh3 = qh[:, 0:NBH * ZB].rearrange("p (b z) -> p b z", z=ZB)
        nc.vector.memset(qh3[:, :, 0:3], 0.0)
        nc.vector.memset(qh[:, NBH * ZB:], 0.0)

        for c in range(NG // 2):  # chunks of 2 w-groups = [96, 2048] cols
            pd = psum.tile([H, 1024], f32, tag="ps")
            pd2 = psum.tile([H, 1024], f32, tag="ps")
            for piece, pt in ((0, pd), (1, pd2)):
                g = 2 * c + piece
                base = 32 * (g % 3)
                nc.tensor.matmul(
                    pt[:, 0:512],
                    xt9g[g][:, i * H:(i + 1) * H],
                    crhs_sb[half][base:base + 9, 0:512],
                    start=True, stop=True,
                )
                nc.tensor.matmul(
                    pt[:, 512:1024],
                    xt9g[g][:, i * H:(i + 1) * H],
                    crhs_sb[half][base:base + 9, 512:1024],
                    start=True, stop=True,
                )
            tt = chunks.tile([H, 2048], f32, tag="t")
            nc.scalar.activation(tt[:, 0:1024], pd[:], AF.Tanh)
            nc.scalar.activation(tt[:, 1024:2048], pd2[:], AF.Tanh)
            kk = chunks.tile([H, 2048], f32, tag="k")
            nc.vector.tensor_tensor(kk[:], tt[:], tt[:], op=OP.mult)
            nc.vector.tensor_scalar(kk[:], kk[:], -0.25, 0.25, op0=OP.mult, op1=OP.add)
            for piece in range(2):
                ph = psum.tile([H, 1024], f32, tag="ps")
                for pp in range(2):
                    nc.tensor.matmul(
                        ph[:, pp * 512:(pp + 1) * 512],
                        band_sb[:],
                        kk[:, piece * 1024 + pp * 512:piece * 1024 + (pp + 1) * 512],
                        start=True, stop=True,
                    )
                # evac: chunk piece covers w-group g = 2c+piece (8 w), all bins
                g = 2 * c + piece
                dst = qh3[:, :, 3 + 8 * g:3 + 8 * g + 8].transpose([0, 2, 1])
                nc.scalar.copy(dst, ph[:].rearrange("p (w b) -> p w b", b=NBH))

        nc.vector.tensor_tensor_scan(
            qh[:], qh[:], qh[:], 0.0, op0=OP.add, op1=OP.bypass
        )
        stripe_store[(i, half)] = (qh, qh3)

    def emit_backend(i):
        qhs = [stripe_store.pop((i, 0)), stripe_store.pop((i, 1))]
        for wc in range(W // 4):
            w0 = 4 * wc
            qt = chunks.tile([H, 4, NB], f32, tag="q")
            for half, (qh, qh3) in enumerate(qhs):
                if w0 + 9 <= ZB:
                    hi = qh3[:, :, w0 + 5:w0 + 9].transpose([0, 2, 1])
                    lo = qh3[:, :, w0:w0 + 4].transpose([0, 2, 1])
                    nc.vector.tensor_tensor(
                        qt[:, :, half * NBH:(half + 1) * NBH], hi, lo,
                        op=OP.subtract,
                    )
                else:
                    for wi in range(4):
                        nc.vector.tensor_tensor(
                            qt[:, wi, half * NBH:(half + 1) * NBH],
                            qh[:, w0 + 5 + wi::ZB][:, 0:NBH],
                            qh[:, w0 + wi::ZB][:, 0:NBH],
                            op=OP.subtract,
                        )
            ltile = chunks.tile([H, 1024], f32, tag="L")
            for j in range(4):
                w = w0 + j
                rcol = rtile[:, i * W + w:i * W + w + 1]
                nc.scalar.activation(
                    ltile[:, j * 256:(j + 1) * 256],
                    qt[:, j, :],
                    AF.Ln,
                    bias=bias_ap(EPS),
                    scale=rcol,
                )
            l3 = ltile[:].rearrange("p (a b) -> p a b", b=NB)
            nc.vector.tensor_tensor(l3, qt[:], l3, op=OP.mult)
            nc.vector.tensor_reduce(
                QL[:, i * W + w0:i * W + w0 + 4],
                l3,
                axis=mybir.AxisListType.X,
                op=OP.add,
            )

    emit_front(0, 0)
    emit_front(0, 1)
    emit_front(1, 0)
    emit_backend(0)
    emit_front(1, 1)
    emit_front(2, 0)
    emit_backend(1)
    emit_front(2, 1)
    emit_backend(2)

    # E = -(r * QL) ; write out
    ent = sm.tile([H, NW], f32)
    nc.vector.tensor_tensor(ent[:], rtile[:], QL[:], op=OP.mult)
    nc.vector.tensor_scalar(ent[:], ent[:], -1.0, None, op0=OP.mult)
    for i in range(NIMG):
        nc.sync.dma_start(ent_d[i], ent[:, i * W:(i + 1) * W])


def _get_compiled():
    if "nc" in _CACHE:
        return _CACHE["nc"]
    from contextlib import ExitStack

    import concourse.tile as tile
    from concourse import bacc, mybir

    f32 = mybir.dt.float32
    nc = bacc.Bacc("TRN2", target_bir_lowering=False, debug=False)
    x_d = nc.dram_tensor("x_sh", [NIMG, H, W], f32, kind="ExternalInput").ap()
    xt_d = nc.dram_tensor("xt_sh", [NIMG, W, H], f32, kind="ExternalInput").ap()
    crhs0_d = nc.dram_tensor("crhs0", [9, WQ * NBH], f32, kind="ExternalInput").ap()
    crhs1_d = nc.dram_tensor("crhs1", [9, WQ * NBH], f32, kind="ExternalInput").ap()
    band_d = nc.dram_tensor("bandh", [H, H], f32, kind="ExternalInput").ap()
    ent_d = nc.dram_tensor("ent", [NIMG, H, W], f32, kind="ExternalOutput").ap()

    with tile.TileContext(nc) as tc:
        with ExitStack() as ctx:
            _emit_kernel(
                nc, tc, ctx, (x_d, xt_d, crhs0_d, crhs1_d, band_d), (ent_d,)
            )
    nc.compile()
    _CACHE["nc"] = nc
    return nc


def make_in_maps(x):
    """x: full [8, 3, 96, 96] -> list of 8 per-core input dicts."""
    x = np.ascontiguousarray(np.asarray(x, dtype=np.float32))
    imgs = x.reshape(NCORES * NIMG, H, W)
    crhs0, crhs1, band = _build_consts()
    in_maps = []
    for c in range(NCORES):
        sh = np.ascontiguousarray(imgs[c * NIMG:(c + 1) * NIMG])
        in_maps.append(
            {
                "x_sh": sh,
                "xt_sh": np.ascontiguousarray(sh.transpose(0, 2, 1)),
                "crhs0": crhs0,
                "crhs1": crhs1,
                "bandh": band,
            }
        )
    return in_maps


def kernel(x):
    """Full inputs in, full outputs out. x: [8, 3, 96, 96] f32."""
    from concourse.bass_utils import run_bass_kernel_spmd

    nc = _get_compiled()
    in_maps = make_in_maps(x)
    res = run_bass_kernel_spmd(nc, in_maps, list(range(NCORES)))
    out = np.stack([res.results[c]["ent"] for c in range(NCORES)])
    return out.reshape(8, 3, H, W).astype(np.float32)



# revision 23
# speedup vs baseline: 2.5863x; 2.5863x over previous
"""Trainium2 Bass kernel for nn_Entropy (histogram_binning): per-pixel Shannon
entropy of a 5x5-window KDE histogram over 256 intensity bins.

Math: k(x,b) = sigmoid'(10(x-b)) = s(1-s), s = sigmoid(10(x-b)).
  q[h,w,b] = 5x5 zero-padded window sum of k;  S = sum_b q (analytic taps);
  E = (S*ln(S+eps) - sum_b q*ln(q+eps)) / (S+eps).

Engine split per (img, bin-half) stripe, layout [96h, (w,b)] b-inner (bf16):
  - z = 10x-10b on TensorE: one K=18 matmul per 8-w chunk; stationary =
    [x_hi^T; x_lo^T; ones; ones] (split-bf16 exact), moving = tiny const
    selector carrying 10*onehot and the -10b bias split (16m + r, both
    bf16-exact).
  - s = Sigmoid (ScalarE, PSUM->SBUF), u = Square(s - 0.5) (ScalarE)
    -> written into a w-padded k-stripe; k = 0.25 - u via one in-place
    4x-mode tensor_scalar (VectorE).
  - BOTH window dims on TensorE: 5 shifted accumulating band matmuls
    (band[h',h] = |h-h'|<=2, bf16) over the padded k-stripe -> q in PSUM.
  - backend: L = Ln(q + eps) (ScalarE), e = q*L (VectorE TT), per-w bin
    reduce (VectorE) -> T; finalize E from T and the analytic S path.
Stripes are software-pipelined (front of stripe i+1 overlaps backend of i).
Sharding: B*C = 24 images, 3 per core across 8 cores; no collectives.
"""

import sys

sys.path.insert(0, "/opt/trn_rl_repo")

import numpy as np

H = 96
W = 96
NB = 256
NBH = 128         # bins per stripe (half)
NIMG = 3
NCORES = 8
EPS = 1e-10
CHW = 8           # w's per chunk
NCH = W // CHW    # 12 chunks per stripe
KBLK = W + 4      # w-blocks in padded k stripe (2 pads each side)
KCOLS = KBLK * NBH
DCOLS = W * NBH   # 12288 data cols per stripe
NW = NIMG * W

_CACHE = {}


def _bf16(a):
    import ml_dtypes

    return np.asarray(a, dtype=ml_dtypes.bfloat16)


def _build_consts():
    import ml_dtypes

    # selector moving constants per bin-half: [26, CHW*NBH]
    movs = []
    for half in range(2):
        c = np.zeros((26, CHW * NBH), dtype=np.float64)
        for j in range(CHW):
            c[j, j * NBH:(j + 1) * NBH] = 10.0
            c[j + 8, j * NBH:(j + 1) * NBH] = 10.0
            c[j + 16, j * NBH:(j + 1) * NBH] = 10.0
        b = np.arange(NBH, dtype=np.float64) + half * NBH
        tb = 10.0 * b
        A = 16.0 * np.floor(tb / 16.0)
        Br = tb - A
        c[24, :] = np.tile(-A, CHW)
        c[25, :] = np.tile(-Br, CHW)
        movs.append(_bf16(c))
    hh = np.arange(H)
    band = (np.abs(hh[:, None] - hh[None, :]) <= 2).astype(np.float32)
    return movs[0], movs[1], _bf16(band), band


def _emit_spath(nc, tc, sm, xall, bandf_sb, bias_ap):
    """Analytic per-pixel KDE mass S summed over the 5x5 window.
    Returns (swin [H, NIMG, W] f32 window-summed S, rtile = 1/(S+EPS))."""
    from concourse import mybir

    f32 = mybir.dt.float32
    i32 = mybir.dt.int32
    AF = mybir.ActivationFunctionType
    OP = mybir.AluOpType

    ni = sm.tile([H, NW], i32)
    nc.vector.tensor_copy(ni[:], xall[:])
    nf = sm.tile([H, NW], f32)
    nc.vector.tensor_copy(nf[:], ni[:])
    u = sm.tile([H, NW], f32)
    nc.vector.tensor_tensor(u[:], xall[:], nf[:], op=OP.subtract)
    taps = (-2, -1, 0, 1, 2)
    sq = {}
    for o in taps:
        v = sm.tile([H, NW], f32, tag=f"v{o}")
        nc.scalar.activation(v[:], u[:], AF.Tanh, bias=bias_ap(-5.0 * o), scale=5.0)
        s2 = sm.tile([H, NW], f32, tag=f"sq{o}")
        nc.scalar.activation(s2[:], v[:], AF.Square)
        sq[o] = s2
    masks = {}
    for o in taps:
        if o == 0:
            continue
        m = sm.tile([H, NW], f32, tag=f"m{o}")
        if o < 0:
            nc.vector.tensor_scalar(m[:], nf[:], float(-o), None, op0=OP.is_ge)
        else:
            nc.vector.tensor_scalar(m[:], nf[:], float(255 - o), None, op0=OP.is_le)
        masks[o] = m
    cnt = sm.tile([H, NW], f32)
    nc.vector.tensor_tensor(cnt[:], masks[-2][:], masks[-1][:], op=OP.add)
    nc.vector.tensor_tensor(cnt[:], cnt[:], masks[1][:], op=OP.add)
    nc.vector.tensor_tensor(cnt[:], cnt[:], masks[2][:], op=OP.add)
    nc.vector.tensor_scalar(cnt[:], cnt[:], 1.0, None, op0=OP.add)
    ssum = sm.tile([H, NW], f32)
    nc.vector.tensor_copy(ssum[:], sq[0][:])
    for o in (-2, -1, 1, 2):
        t_m = sm.tile([H, NW], f32, tag=f"tm{o}")
        nc.vector.tensor_tensor(t_m[:], masks[o][:], sq[o][:], op=OP.mult)
        nc.vector.tensor_tensor(ssum[:], ssum[:], t_m[:], op=OP.add)
    spix = sm.tile([H, NW], f32)
    nc.vector.tensor_tensor(spix[:], cnt[:], ssum[:], op=OP.subtract)
    nc.vector.tensor_scalar(spix[:], spix[:], 0.25, None, op0=OP.mult)
    return spix


def _emit_kernel(nc, tc, ctx, ins, outs):
    from concourse import mybir

    f32 = mybir.dt.float32
    bf16 = mybir.dt.bfloat16
    AF = mybir.ActivationFunctionType
    OP = mybir.AluOpType

    x_d, xg_d, mov0_d, mov1_d, band_d, bandf_d = ins
    (ent_d,) = outs

    consts = ctx.enter_context(tc.tile_pool(name="consts", bufs=1))
    sm = ctx.enter_context(tc.tile_pool(name="sm", bufs=1))
    kpool = ctx.enter_context(tc.tile_pool(name="kpool", bufs=3))
    spool = ctx.enter_context(tc.tile_pool(name="spool", bufs=3))
    upool = ctx.enter_context(tc.tile_pool(name="upool", bufs=3))
    lpool = ctx.enter_context(tc.tile_pool(name="lpool", bufs=3))
    epool = ctx.enter_context(tc.tile_pool(name="epool", bufs=3))
    zpsum = ctx.enter_context(tc.tile_pool(name="zpsum", bufs=2, space="PSUM"))
    qpsum = ctx.enter_context(tc.tile_pool(name="qpsum", bufs=2, space="PSUM"))

    # ---- constants / inputs ----
    # selector replicated at partition bases 0/32/64 to match stationary base
    mov_sb = []
    for half, md in ((0, mov0_d), (1, mov1_d)):
        t = consts.tile([90, CHW * NBH], bf16, tag=f"mov{half}")
        for k3 in range(3):
            nc.sync.dma_start(t[32 * k3:32 * k3 + 26, :], md[:])
        mov_sb.append(t)
    band_sb = consts.tile([H, H], bf16)
    nc.sync.dma_start(band_sb[:], band_d[:])
    bandf_sb = consts.tile([H, H], f32)
    nc.sync.dma_start(bandf_sb[:], bandf_d[:])

    xall = consts.tile([H, NW], f32)
    for i in range(NIMG):
        nc.sync.dma_start(xall[:, i * W:(i + 1) * W], x_d[i])

    # stationary groups: [26 rows: x^T 3-way bf16 split (8 w each); ones x2]
    # packed 3 groups per tile at partition offsets 0/32/64 (LDW constraint),
    # host-assembled and shipped whole
    xg_ap = []  # per group g: (tile, row base)
    for tg in range(4):
        gt = consts.tile([90, NIMG * H], bf16, tag=f"xg{tg}")
        nc.sync.dma_start(gt[:], xg_d[tg])
        for k3 in range(3):
            xg_ap.append((gt, 32 * k3))

    bias_tiles = {}

    def bias_ap(val):
        if val not in bias_tiles:
            t = consts.tile([H, 1], f32, tag=f"bias{val}")
            nc.vector.memset(t[:], val)
            bias_tiles[val] = t
        return bias_tiles[val][:]

    # =====================  analytic S path  =====================
    spix = _emit_spath(nc, tc, sm, xall, bandf_sb, bias_ap)
    ps_s = zpsum.tile([H, 1024], f32, tag="z")
    nc.tensor.matmul(ps_s[:, 0:NW], bandf_sb[:], spix[:], start=True, stop=True)
    sh = sm.tile([H, NW], f32)
    nc.scalar.copy(sh[:], ps_s[:, 0:NW])
    shp = sm.tile([H, NIMG, W + 4], f32)
    nc.vector.memset(shp[:], 0.0)
    for i in range(NIMG):
        nc.vector.tensor_copy(shp[:, i, 2:2 + W], sh[:, i * W:(i + 1) * W])
    swin = sm.tile([H, NIMG, W], f32)
    nc.vector.tensor_tensor(swin[:], shp[:, :, 0:W], shp[:, :, 1:1 + W], op=OP.add)
    for j in (2, 3, 4):
        nc.vector.tensor_tensor(swin[:], swin[:], shp[:, :, j:j + W], op=OP.add)
    sw_flat = swin[:].rearrange("p a b -> p (a b)")
    rtile = sm.tile([H, NW], f32)
    nc.vector.tensor_scalar(rtile[:], sw_flat, EPS, None, op0=OP.add)
    nc.vector.reciprocal(rtile[:], rtile[:])
    lnS = sm.tile([H, NW], f32)
    nc.scalar.activation(lnS[:], sw_flat, AF.Ln, bias=bias_ap(EPS))
    slns = sm.tile([H, NW], f32)
    nc.vector.tensor_tensor(slns[:], sw_flat, lnS[:], op=OP.mult)

    # =====================  main pipeline  =====================
    # T accumulator per half
    QL0 = sm.tile([H, NW], f32, tag="QL0")
    QL1 = sm.tile([H, NW], f32, tag="QL1")
    QL = [QL0, QL1]
    stripe_store = {}

    def front(s):
        i, half = s // 2, s % 2
        kt = kpool.tile([H, KCOLS], bf16, tag="kt")
        nc.vector.memset(kt[:, 0:2 * NBH], 0.0)
        nc.vector.memset(kt[:, KCOLS - 2 * NBH:], 0.0)
        for c in range(NCH):
            gt, base = xg_ap[c]
            zp = zpsum.tile([H, 1024], f32, tag="z")
            for p in range(2):
                nc.tensor.matmul(
                    zp[:, 512 * p:512 * (p + 1)],
                    gt[base:base + 26, i * H:(i + 1) * H],
                    mov_sb[half][base:base + 26, 512 * p:512 * (p + 1)],
                    start=True, stop=True,
                )
            sc = spool.tile([H, 1024], f32, tag="s")
            nc.scalar.activation(sc[:], zp[:], AF.Sigmoid)
            uc = upool.tile([H, 1024], f32, tag="u")
            nc.scalar.activation(uc[:], sc[:], AF.Square, bias=bias_ap(-0.5))
            # k = 0.25 - u (f32 in, bf16 out: only relative rounding on k)
            nc.vector.tensor_scalar(
                kt[:, (2 + c * CHW) * NBH:(2 + (c + 1) * CHW) * NBH],
                uc[:],
                -1.0, 0.25, op0=OP.mult, op1=OP.add,
            )
        stripe_store[s] = kt

    def backend(s):
        i, half = s // 2, s % 2
        kt = stripe_store.pop(s)
        for c in range(NCH):
            qp = qpsum.tile([H, 1024], f32, tag="q")
            for p in range(2):
                for s0 in range(5):
                    base_col = (c * CHW + s0) * NBH + 512 * p
                    nc.tensor.matmul(
                        qp[:, 512 * p:512 * (p + 1)],
                        band_sb[:],
                        kt[:, base_col:base_col + 512],
                        start=(s0 == 0), stop=(s0 == 4),
                    )
            lc = lpool.tile([H, 1024], f32, tag="L")
            nc.scalar.activation(lc[:], qp[:], AF.Ln, bias=bias_ap(EPS))
            ec = epool.tile([H, 1024], f32, tag="e")
            nc.vector.tensor_tensor(ec[:], qp[:], lc[:], op=OP.mult)
            nc.vector.tensor_reduce(
                QL[half][:, i * W + c * CHW:i * W + (c + 1) * CHW],
                ec[:].rearrange("p (a b) -> p a b", b=NBH),
                axis=mybir.AxisListType.X,
                op=OP.add,
            )

    front(0)
    front(1)
    for s in range(2, 6):
        front(s)
        backend(s - 2)
    backend(4)
    backend(5)

    # E = rtile * (S*ln(S+eps) - T0 - T1)
    ent = sm.tile([H, NW], f32)
    nc.vector.tensor_tensor(ent[:], slns[:], QL[0][:], op=OP.subtract)
    nc.vector.tensor_tensor(ent[:], ent[:], QL[1][:], op=OP.subtract)
    nc.vector.tensor_tensor(ent[:], ent[:], rtile[:], op=OP.mult)
    for i in range(NIMG):
        nc.sync.dma_start(ent_d[i], ent[:, i * W:(i + 1) * W])


def _get_compiled():
    if "nc" in _CACHE:
        return _CACHE["nc"]
    from contextlib import ExitStack

    import concourse.tile as tile
    from concourse import bacc, mybir

    f32 = mybir.dt.float32
    bf16 = mybir.dt.bfloat16
    nc = bacc.Bacc("TRN2", target_bir_lowering=False, debug=False)
    x_d = nc.dram_tensor("x_sh", [NIMG, H, W], f32, kind="ExternalInput").ap()
    xg_d = nc.dram_tensor("xg", [4, 90, NIMG * H], bf16, kind="ExternalInput").ap()
    mov0_d = nc.dram_tensor("mov0", [26, CHW * NBH], bf16, kind="ExternalInput").ap()
    mov1_d = nc.dram_tensor("mov1", [26, CHW * NBH], bf16, kind="ExternalInput").ap()
    band_d = nc.dram_tensor("bandb", [H, H], bf16, kind="ExternalInput").ap()
    bandf_d = nc.dram_tensor("bandf", [H, H], f32, kind="ExternalInput").ap()
    ent_d = nc.dram_tensor("ent", [NIMG, H, W], f32, kind="ExternalOutput").ap()

    with tile.TileContext(nc) as tc:
        with ExitStack() as ctx:
            _emit_kernel(
                nc, tc, ctx,
                (x_d, xg_d, mov0_d, mov1_d, band_d, bandf_d),
                (ent_d,),
            )
    nc.compile()
    _CACHE["nc"] = nc
    return nc


def make_in_maps(x):
    """x: full [8, 3, 96, 96] -> list of 8 per-core input dicts."""
    import ml_dtypes

    x = np.ascontiguousarray(np.asarray(x, dtype=np.float32))
    imgs = x.reshape(NCORES * NIMG, H, W)
    mov0, mov1, bandb, bandf = _build_consts()
    in_maps = []
    for cidx in range(NCORES):
        sh = np.ascontiguousarray(imgs[cidx * NIMG:(cidx + 1) * NIMG])
        xt = sh.transpose(0, 2, 1).transpose(1, 0, 2).reshape(W, NIMG * H)
        # xt[w, i*H+h] = x[i, h, w]
        xt_hi = np.asarray(_bf16(xt), dtype=np.float32)
        xt_mid = np.asarray(_bf16(xt - xt_hi), dtype=np.float32)
        xt_lo = xt - xt_hi - xt_mid
        xg = np.zeros((4, 90, NIMG * H), dtype=np.float32)
        for g in range(12):
            tg, base = g // 3, 32 * (g % 3)
            xg[tg, base:base + 8] = xt_hi[8 * g:8 * g + 8]
            xg[tg, base + 8:base + 16] = xt_mid[8 * g:8 * g + 8]
            xg[tg, base + 16:base + 24] = xt_lo[8 * g:8 * g + 8]
            xg[tg, base + 24:base + 26] = 1.0
        in_maps.append(
            {
                "x_sh": sh,
                "xg": _bf16(xg),
                "mov0": mov0,
                "mov1": mov1,
                "bandb": bandb,
                "bandf": bandf,
            }
        )
    return in_maps


def kernel(x):
    """Full inputs in, full outputs out. x: [8, 3, 96, 96] f32."""
    from concourse.bass_utils import run_bass_kernel_spmd

    nc = _get_compiled()
    in_maps = make_in_maps(x)
    res = run_bass_kernel_spmd(nc, in_maps, list(range(NCORES)))
    out = np.stack([res.results[c]["ent"] for c in range(NCORES)])
    return out.reshape(8, 3, H, W).astype(np.float32)


# revision 28
# speedup vs baseline: 2.7959x; 1.0810x over previous
"""Trainium2 Bass kernel for nn_Entropy (histogram_binning): per-pixel Shannon
entropy of a 5x5-window KDE histogram over 256 intensity bins.

Math: k(x,b) = sigmoid'(10(x-b)) = s(1-s), s = sigmoid(10(x-b)).
  q[h,w,b] = 5x5 zero-padded window sum of k;  S = sum_b q (analytic taps);
  E = (S*ln(S+eps) - sum_b q*ln(q+eps)) / (S+eps).

Engine split per (img, bin-half) stripe, layout [96h, (w,b)] b-inner (bf16):
  - z = 10x-10b on TensorE: one K=18 matmul per 8-w chunk; stationary =
    [x_hi^T; x_lo^T; ones; ones] (split-bf16 exact), moving = tiny const
    selector carrying 10*onehot and the -10b bias split (16m + r, both
    bf16-exact).
  - s = Sigmoid (ScalarE, PSUM->SBUF), u = Square(s - 0.5) (ScalarE)
    -> written into a w-padded k-stripe; k = 0.25 - u via one in-place
    4x-mode tensor_scalar (VectorE).
  - BOTH window dims on TensorE: 5 shifted accumulating band matmuls
    (band[h',h] = |h-h'|<=2, bf16) over the padded k-stripe -> q in PSUM.
  - backend: L = Ln(q + eps) (ScalarE), e = q*L (VectorE TT), per-w bin
    reduce (VectorE) -> T; finalize E from T and the analytic S path.
Stripes are software-pipelined (front of stripe i+1 overlaps backend of i).
Sharding: B*C = 24 images, 3 per core across 8 cores; no collectives.
"""

import sys

sys.path.insert(0, "/opt/trn_rl_repo")

import numpy as np

H = 96
W = 96
NB = 256
NBH = 128         # bins per stripe (half)
NIMG = 3
NCORES = 8
EPS = 1e-10
CHW = 8           # w's per chunk
NCH = W // CHW    # 12 chunks per stripe
KBLK = W + 4      # w-blocks in padded k stripe (2 pads each side)
KCOLS = KBLK * NBH
DCOLS = W * NBH   # 12288 data cols per stripe
NW = NIMG * W
STT_STRIPES = frozenset((0, 2, 4))  # stripes whose k is built on DVE (else ScalarE)

_CACHE = {}


def _bf16(a):
    import ml_dtypes

    return np.asarray(a, dtype=ml_dtypes.bfloat16)


def _build_consts():
    import ml_dtypes

    # selector moving constants per bin-half: [26, CHW*NBH]
    movs = []
    for half in range(2):
        c = np.zeros((26, CHW * NBH), dtype=np.float64)
        for j in range(CHW):
            c[j, j * NBH:(j + 1) * NBH] = 10.0
            c[j + 8, j * NBH:(j + 1) * NBH] = 10.0
            c[j + 16, j * NBH:(j + 1) * NBH] = 10.0
        b = np.arange(NBH, dtype=np.float64) + half * NBH
        tb = 10.0 * b
        A = 16.0 * np.floor(tb / 16.0)
        Br = tb - A
        c[24, :] = np.tile(-A, CHW)
        c[25, :] = np.tile(-Br, CHW)
        movs.append(_bf16(c))
    hh = np.arange(H)
    band = (np.abs(hh[:, None] - hh[None, :]) <= 2).astype(np.float32)
    # main-path band is NEGATED: k-tile holds -k = (s-1)*s from the gpsimd
    # scalar_tensor_tensor, so q = sum((-1) * (-k)) = sum(k)
    return movs[0], movs[1], _bf16(-band), band


def _emit_spath(nc, tc, sm, xall, bandf_sb, bias_ap):
    """Analytic per-pixel KDE mass S summed over the 5x5 window.
    Returns (swin [H, NIMG, W] f32 window-summed S, rtile = 1/(S+EPS))."""
    from concourse import mybir

    f32 = mybir.dt.float32
    i32 = mybir.dt.int32
    AF = mybir.ActivationFunctionType
    OP = mybir.AluOpType

    ni = sm.tile([H, NW], i32)
    nc.vector.tensor_copy(ni[:], xall[:])
    nf = sm.tile([H, NW], f32)
    nc.vector.tensor_copy(nf[:], ni[:])
    u = sm.tile([H, NW], f32)
    nc.vector.tensor_tensor(u[:], xall[:], nf[:], op=OP.subtract)
    taps = (-2, -1, 0, 1, 2)
    sq = {}
    for o in taps:
        v = sm.tile([H, NW], f32, tag=f"v{o}")
        nc.scalar.activation(v[:], u[:], AF.Tanh, bias=bias_ap(-5.0 * o), scale=5.0)
        s2 = sm.tile([H, NW], f32, tag=f"sq{o}")
        nc.scalar.activation(s2[:], v[:], AF.Square)
        sq[o] = s2
    masks = {}
    for o in taps:
        if o == 0:
            continue
        m = sm.tile([H, NW], f32, tag=f"m{o}")
        if o < 0:
            nc.vector.tensor_scalar(m[:], nf[:], float(-o), None, op0=OP.is_ge)
        else:
            nc.vector.tensor_scalar(m[:], nf[:], float(255 - o), None, op0=OP.is_le)
        masks[o] = m
    cnt = sm.tile([H, NW], f32)
    nc.vector.tensor_tensor(cnt[:], masks[-2][:], masks[-1][:], op=OP.add)
    nc.vector.tensor_tensor(cnt[:], cnt[:], masks[1][:], op=OP.add)
    nc.vector.tensor_tensor(cnt[:], cnt[:], masks[2][:], op=OP.add)
    nc.vector.tensor_scalar(cnt[:], cnt[:], 1.0, None, op0=OP.add)
    ssum = sm.tile([H, NW], f32)
    nc.vector.tensor_copy(ssum[:], sq[0][:])
    for o in (-2, -1, 1, 2):
        t_m = sm.tile([H, NW], f32, tag=f"tm{o}")
        nc.vector.tensor_tensor(t_m[:], masks[o][:], sq[o][:], op=OP.mult)
        nc.vector.tensor_tensor(ssum[:], ssum[:], t_m[:], op=OP.add)
    spix = sm.tile([H, NW], f32)
    nc.vector.tensor_tensor(spix[:], cnt[:], ssum[:], op=OP.subtract)
    nc.vector.tensor_scalar(spix[:], spix[:], 0.25, None, op0=OP.mult)
    return spix


def _emit_kernel(nc, tc, ctx, ins, outs):
    from concourse import mybir

    f32 = mybir.dt.float32
    bf16 = mybir.dt.bfloat16
    AF = mybir.ActivationFunctionType
    OP = mybir.AluOpType

    x_d, xg_d, mov0_d, mov1_d, band_d, bandf_d = ins
    (ent_d,) = outs

    consts = ctx.enter_context(tc.tile_pool(name="consts", bufs=1))
    sm = ctx.enter_context(tc.tile_pool(name="sm", bufs=1))
    kpool = ctx.enter_context(tc.tile_pool(name="kpool", bufs=3))
    spool = ctx.enter_context(tc.tile_pool(name="spool", bufs=3))
    upool = ctx.enter_context(tc.tile_pool(name="upool", bufs=3))
    lpool = ctx.enter_context(tc.tile_pool(name="lpool", bufs=3))
    epool = ctx.enter_context(tc.tile_pool(name="epool", bufs=3))
    zpsum = ctx.enter_context(tc.tile_pool(name="zpsum", bufs=2, space="PSUM"))
    qpsum = ctx.enter_context(tc.tile_pool(name="qpsum", bufs=2, space="PSUM"))

    # ---- constants / inputs ----
    # selector replicated at partition bases 0/32/64 to match stationary base
    mov_sb = []
    for half, md in ((0, mov0_d), (1, mov1_d)):
        t = consts.tile([90, CHW * NBH], bf16, tag=f"mov{half}")
        for k3 in range(3):
            nc.sync.dma_start(t[32 * k3:32 * k3 + 26, :], md[:])
        mov_sb.append(t)
    band_sb = consts.tile([H, H], bf16)
    nc.sync.dma_start(band_sb[:], band_d[:])
    bandf_sb = consts.tile([H, H], f32)
    nc.sync.dma_start(bandf_sb[:], bandf_d[:])

    xall = consts.tile([H, NW], f32)
    for i in range(NIMG):
        nc.sync.dma_start(xall[:, i * W:(i + 1) * W], x_d[i])

    # stationary groups: [26 rows: x^T 3-way bf16 split (8 w each); ones x2]
    # packed 3 groups per tile at partition offsets 0/32/64 (LDW constraint),
    # host-assembled and shipped whole
    xg_ap = []  # per group g: (tile, row base)
    for tg in range(4):
        gt = consts.tile([90, NIMG * H], bf16, tag=f"xg{tg}")
        nc.sync.dma_start(gt[:], xg_d[tg])
        for k3 in range(3):
            xg_ap.append((gt, 32 * k3))

    bias_tiles = {}

    def bias_ap(val):
        if val not in bias_tiles:
            t = consts.tile([H, 1], f32, tag=f"bias{val}")
            nc.vector.memset(t[:], val)
            bias_tiles[val] = t
        return bias_tiles[val][:]

    # =====================  analytic S path  =====================
    spix = _emit_spath(nc, tc, sm, xall, bandf_sb, bias_ap)
    ps_s = zpsum.tile([H, 1024], f32, tag="z")
    nc.tensor.matmul(ps_s[:, 0:NW], bandf_sb[:], spix[:], start=True, stop=True)
    sh = sm.tile([H, NW], f32)
    nc.scalar.copy(sh[:], ps_s[:, 0:NW])
    shp = sm.tile([H, NIMG, W + 4], f32)
    nc.vector.memset(shp[:], 0.0)
    for i in range(NIMG):
        nc.vector.tensor_copy(shp[:, i, 2:2 + W], sh[:, i * W:(i + 1) * W])
    swin = sm.tile([H, NIMG, W], f32)
    nc.vector.tensor_tensor(swin[:], shp[:, :, 0:W], shp[:, :, 1:1 + W], op=OP.add)
    for j in (2, 3, 4):
        nc.vector.tensor_tensor(swin[:], swin[:], shp[:, :, j:j + W], op=OP.add)
    sw_flat = swin[:].rearrange("p a b -> p (a b)")
    rtile = sm.tile([H, NW], f32)
    nc.vector.tensor_scalar(rtile[:], sw_flat, EPS, None, op0=OP.add)
    nc.vector.reciprocal(rtile[:], rtile[:])
    lnS = sm.tile([H, NW], f32)
    nc.scalar.activation(lnS[:], sw_flat, AF.Ln, bias=bias_ap(EPS))
    slns = sm.tile([H, NW], f32)
    nc.vector.tensor_tensor(slns[:], sw_flat, lnS[:], op=OP.mult)

    # =====================  main pipeline  =====================
    # T accumulator per half
    QL0 = sm.tile([H, NW], f32, tag="QL0")
    QL1 = sm.tile([H, NW], f32, tag="QL1")
    QL = [QL0, QL1]
    stripe_store = {}

    def front(s):
        i, half = s // 2, s % 2
        kt = kpool.tile([H, KCOLS], bf16, tag="kt")
        nc.vector.memset(kt[:, 0:2 * NBH], 0.0)
        nc.vector.memset(kt[:, KCOLS - 2 * NBH:], 0.0)
        for c in range(NCH):
            gt, base = xg_ap[c]
            zp = zpsum.tile([H, 1024], f32, tag="z")
            for p in range(2):
                nc.tensor.matmul(
                    zp[:, 512 * p:512 * (p + 1)],
                    gt[base:base + 26, i * H:(i + 1) * H],
                    mov_sb[half][base:base + 26, 512 * p:512 * (p + 1)],
                    start=True, stop=True,
                )
            sc = spool.tile([H, 1024], f32, tag="s")
            nc.scalar.activation(sc[:], zp[:], AF.Sigmoid)
            kdst = kt[:, (2 + c * CHW) * NBH:(2 + (c + 1) * CHW) * NBH]
            if s in STT_STRIPES:
                # -k = (s - 1) * s in one DVE pass (f32 in, bf16 out)
                nc.vector.scalar_tensor_tensor(
                    kdst, sc[:], 1.0, sc[:], op0=OP.subtract, op1=OP.mult,
                )
            else:
                # u = (s - 0.5)^2 on ScalarE, then -k = u - 0.25 on DVE (2x)
                uc = upool.tile([H, 1024], f32, tag="u")
                nc.scalar.activation(uc[:], sc[:], AF.Square, bias=bias_ap(-0.5))
                nc.vector.tensor_scalar(
                    kdst, uc[:], 1.0, -0.25, op0=OP.mult, op1=OP.add,
                )
        stripe_store[s] = kt

    def backend(s):
        i, half = s // 2, s % 2
        kt = stripe_store.pop(s)
        for c in range(NCH):
            qp = qpsum.tile([H, 1024], f32, tag="q")
            for p in range(2):
                for s0 in range(5):
                    base_col = (c * CHW + s0) * NBH + 512 * p
                    nc.tensor.matmul(
                        qp[:, 512 * p:512 * (p + 1)],
                        band_sb[:],
                        kt[:, base_col:base_col + 512],
                        start=(s0 == 0), stop=(s0 == 4),
                    )
            lc = lpool.tile([H, 1024], f32, tag="L")
            nc.scalar.activation(lc[:], qp[:], AF.Ln, bias=bias_ap(EPS))
            ec = epool.tile([H, 1024], f32, tag="e")
            nc.vector.tensor_tensor(ec[:], qp[:], lc[:], op=OP.mult)
            nc.vector.tensor_reduce(
                QL[half][:, i * W + c * CHW:i * W + (c + 1) * CHW],
                ec[:].rearrange("p (a b) -> p a b", b=NBH),
                axis=mybir.AxisListType.X,
                op=OP.add,
            )

    front(0)
    front(1)
    for s in range(2, 6):
        front(s)
        backend(s - 2)
    backend(4)
    backend(5)

    # E = rtile * (S*ln(S+eps) - T0 - T1)
    ent = sm.tile([H, NW], f32)
    nc.vector.tensor_tensor(ent[:], slns[:], QL[0][:], op=OP.subtract)
    nc.vector.tensor_tensor(ent[:], ent[:], QL[1][:], op=OP.subtract)
    nc.vector.tensor_tensor(ent[:], ent[:], rtile[:], op=OP.mult)
    for i in range(NIMG):
        nc.sync.dma_start(ent_d[i], ent[:, i * W:(i + 1) * W])


def _get_compiled():
    if "nc" in _CACHE:
        return _CACHE["nc"]
    from contextlib import ExitStack

    import concourse.tile as tile
    from concourse import bacc, mybir

    f32 = mybir.dt.float32
    bf16 = mybir.dt.bfloat16
    nc = bacc.Bacc("TRN2", target_bir_lowering=False, debug=False)
    x_d = nc.dram_tensor("x_sh", [NIMG, H, W], f32, kind="ExternalInput").ap()
    xg_d = nc.dram_tensor("xg", [4, 90, NIMG * H], bf16, kind="ExternalInput").ap()
    mov0_d = nc.dram_tensor("mov0", [26, CHW * NBH], bf16, kind="ExternalInput").ap()
    mov1_d = nc.dram_tensor("mov1", [26, CHW * NBH], bf16, kind="ExternalInput").ap()
    band_d = nc.dram_tensor("bandb", [H, H], bf16, kind="ExternalInput").ap()
    bandf_d = nc.dram_tensor("bandf", [H, H], f32, kind="ExternalInput").ap()
    ent_d = nc.dram_tensor("ent", [NIMG, H, W], f32, kind="ExternalOutput").ap()

    with tile.TileContext(nc) as tc:
        with ExitStack() as ctx:
            _emit_kernel(
                nc, tc, ctx,
                (x_d, xg_d, mov0_d, mov1_d, band_d, bandf_d),
                (ent_d,),
            )
    nc.compile()
    _CACHE["nc"] = nc
    return nc


def make_in_maps(x):
    """x: full [8, 3, 96, 96] -> list of 8 per-core input dicts."""
    import ml_dtypes

    x = np.ascontiguousarray(np.asarray(x, dtype=np.float32))
    imgs = x.reshape(NCORES * NIMG, H, W)
    mov0, mov1, bandb, bandf = _build_consts()
    in_maps = []
    for cidx in range(NCORES):
        sh = np.ascontiguousarray(imgs[cidx * NIMG:(cidx + 1) * NIMG])
        xt = sh.transpose(0, 2, 1).transpose(1, 0, 2).reshape(W, NIMG * H)
        # xt[w, i*H+h] = x[i, h, w]
        xt_hi = np.asarray(_bf16(xt), dtype=np.float32)
        xt_mid = np.asarray(_bf16(xt - xt_hi), dtype=np.float32)
        xt_lo = xt - xt_hi - xt_mid
        xg = np.zeros((4, 90, NIMG * H), dtype=np.float32)
        for g in range(12):
            tg, base = g // 3, 32 * (g % 3)
            xg[tg, base:base + 8] = xt_hi[8 * g:8 * g + 8]
            xg[tg, base + 8:base + 16] = xt_mid[8 * g:8 * g + 8]
            xg[tg, base + 16:base + 24] = xt_lo[8 * g:8 * g + 8]
            xg[tg, base + 24:base + 26] = 1.0
        in_maps.append(
            {
                "x_sh": sh,
                "xg": _bf16(xg),
                "mov0": mov0,
                "mov1": mov1,
                "bandb": bandb,
                "bandf": bandf,
            }
        )
    return in_maps


def kernel(x):
    """Full inputs in, full outputs out. x: [8, 3, 96, 96] f32."""
    from concourse.bass_utils import run_bass_kernel_spmd

    nc = _get_compiled()
    in_maps = make_in_maps(x)
    res = run_bass_kernel_spmd(nc, in_maps, list(range(NCORES)))
    out = np.stack([res.results[c]["ent"] for c in range(NCORES)])
    return out.reshape(8, 3, H, W).astype(np.float32)
